# revision 11
# baseline (speedup 1.0000x reference)
"""Muskingum-Cunge river routing on a complete binary tree (depth 13, N=8191,
T=2048) — Trainium2 Bass kernel.

Reformulation: the reference's per-timestep leaves->root level sweep is a 2D
dependency (t, level): node at (t, l) needs (t, l+1) [children, same t] and
(t-1, l) [own state]. A wavefront over k = t + (12 - l) makes every superstep
one uniform elementwise MC update over all 8191 nodes plus a parent<-child
pair-sum of the previous superstep's outflows. 2060 supersteps total, fully
sequential; all parallelism is inside a superstep, so a single NeuronCore
running [128, 64] tiles is the right shape.

Layout ([128 partitions, 64 free], node = 0-based heap index):
  - 128 subtrees rooted at the 128 level-7 nodes: partition = subtree,
    f in [0, 62] = local heap index (local children 2m+1, 2m+2 stay in the
    same partition -> the pair-sum for levels 7..11 is ONE stride-2 vector add)
  - top tree (levels 0..6, 127 nodes): (p = heap idx, f = 63). Its pair-sums
    cross partitions -> two tiny PE matmuls with 0/1 weights accumulate
    up[i] = Q[2i+1] + Q[2i+2] into PSUM column 63.

Math (per node, per superstep), with per-node constants precomputed on host:
  Qr2 = max(I_new + O_old, 2e-3)            # = 2*Qr
  lq2 = ln(Qr2)
  [K | t] = exp(lq2 * [a1n | a3] + [P4 | P3])   # K = L/c, t = Qr/(2 w S c L)
  Xc  = min(t, 0.5)                          # X = 0.5 - Xc
  Y   = 2*K*Xc
  den = Y + dt + K ;  n1 = Y + dt - K        # n1 = dt - 2KX
  O'  = relu(O_old + (n1*(I_new - I_old) + 2dt*(I_old - O_old)) / den)
which is algebraically identical to the reference's C1/C2/C3 blend.
"""

import os
import sys

import numpy as np

for _p in ("/opt/trn_rl_repo", "/root/.axon_site/_ro/trn_rl_repo"):
    if os.path.isdir(_p) and _p not in sys.path:
        sys.path.insert(0, _p)

DEPTH = 13
N = 2**DEPTH - 1            # 8191
T = 2048
NSS = T + DEPTH - 1         # 2060 supersteps
CH = 32                     # lat diagonals per DMA chunk
NSS_PAD = ((NSS + CH - 1) // CH) * CH

_F32 = np.float32


def _build_layout():
    """0-based heap node -> (p, f) in the [128, 64] tile, + level."""
    idx = np.arange(1, N + 1, dtype=np.int64)          # 1-based
    lvl = (np.log2(idx)).astype(np.int64)
    # exact bit_length - 1
    lvl = np.frexp(idx.astype(np.float64))[1] - 1
    p = np.empty(N, np.int64)
    f = np.empty(N, np.int64)
    top = lvl <= 6
    p[top] = idx[top] - 1
    f[top] = 63
    for l in range(7, DEPTH):
        m = lvl == l
        j = idx[m]
        d = l - 7
        ja = j >> d
        w = (1 << d) + (j - (ja << d))                 # local 1-based heap
        p[m] = ja - 128
        f[m] = w - 1
    return p, f, lvl


def _host_precompute(inputs):
    lat = np.ascontiguousarray(np.asarray(inputs["lateral_inflows"], _F32))
    n = np.asarray(inputs["manning_n"], _F32).astype(np.float64)
    L = np.asarray(inputs["lengths"], _F32).astype(np.float64)
    S = np.asarray(inputs["slopes"], _F32).astype(np.float64)
    wc = np.asarray(inputs["width_coefs"], _F32).astype(np.float64)
    we = np.asarray(inputs["width_exps"], _F32).astype(np.float64)
    dc = np.asarray(inputs["depth_coefs"], _F32).astype(np.float64)
    de = np.asarray(inputs["depth_exps"], _F32).astype(np.float64)

    p, f, lvl = _build_layout()
    col = p * 64 + f

    c0 = (5.0 / 3.0) * dc ** (2.0 / 3.0) * np.sqrt(S) / n
    a1n = -(2.0 / 3.0) * de
    a3 = 1.0 - we - (2.0 / 3.0) * de
    ln_half = np.log(0.5)
    P4 = np.log(L / c0) + a1n * ln_half
    P3 = np.log(0.5 / (wc * S * L * c0)) + a3 * ln_half

    def scat(vals):
        flat = np.zeros(128 * 64, _F32)
        flat[col] = vals.astype(_F32)
        return flat.reshape(128, 64)

    ACAT = np.concatenate([scat(a1n), scat(a3)], axis=1)      # [128, 128]
    PCAT = np.concatenate([scat(P4), scat(P3)], axis=1)       # [128, 128]

    LD = np.zeros((NSS_PAD, 128 * 64), _F32)
    for l in range(DEPTH):
        s, e = 2**l - 1, 2 ** (l + 1) - 1
        LD[12 - l: 12 - l + T, col[s:e]] = lat[:, s:e]

    Amm = np.zeros((128, 127), _F32)
    Bmm = np.zeros((128, 127), _F32)
    for i in range(63):                   # parents inside top tree (lvl 0..5)
        Amm[2 * i + 1, i] = 1.0
        Amm[2 * i + 2, i] = 1.0
    for i in range(63, 127):              # level-6 parents; children = subtree roots
        Bmm[2 * (i - 63), i] = 1.0
        Bmm[2 * (i - 63) + 1, i] = 1.0
    return LD, ACAT, PCAT, Amm, Bmm


def _build_bass(nss, dtf, wbufs=2, gps_d=False, dve_recip=False, split_u=False):
    """Build the single-core Bass program for `nss` supersteps."""
    from contextlib import ExitStack

    import concourse.bass as bass
    import concourse.tile as tile
    from concourse import bacc, mybir

    f32 = mybir.dt.float32
    OP = mybir.AluOpType
    AF = mybir.ActivationFunctionType

    nss_pad = ((nss + CH - 1) // CH) * CH
    n_chunks = nss_pad // CH

    nc = bacc.Bacc("TRN2", target_bir_lowering=False, debug=False,
                   num_devices=1)
    ld_d = nc.dram_tensor("ld", [nss_pad, 8192], f32, kind="ExternalInput").ap()
    acat_d = nc.dram_tensor("acat", [128, 128], f32, kind="ExternalInput").ap()
    pcat_d = nc.dram_tensor("pcat", [128, 128], f32, kind="ExternalInput").ap()
    amm_d = nc.dram_tensor("amm", [128, 127], f32, kind="ExternalInput").ap()
    bmm_d = nc.dram_tensor("bmm", [128, 127], f32, kind="ExternalInput").ap()
    nout = nss - 12
    out_d = nc.dram_tensor("out", [1, nout], f32, kind="ExternalOutput").ap()

    with tile.TileContext(nc) as tc, ExitStack() as ctx:
        const = ctx.enter_context(tc.tile_pool(name="const", bufs=1))
        state = ctx.enter_context(tc.tile_pool(name="state", bufs=max(2, wbufs)))
        work = ctx.enter_context(tc.tile_pool(name="work", bufs=wbufs))
        psum = ctx.enter_context(tc.tile_pool(name="psum", bufs=1, space="PSUM"))
        latp = ctx.enter_context(tc.tile_pool(name="lat", bufs=2))

        acat = const.tile([128, 128], f32)
        nc.sync.dma_start(acat[:], acat_d)
        pcat = const.tile([128, 128], f32)
        nc.sync.dma_start(pcat[:], pcat_d)
        amm = const.tile([128, 127], f32)
        nc.sync.dma_start(amm[:], amm_d)
        bmm = const.tile([128, 127], f32)
        nc.sync.dma_start(bmm[:], bmm_d)
        outbuf = const.tile([1, nss], f32)

        if split_u:
            U = const.tile([128, 64], f32)       # SBUF: pairsum + zeros
            nc.vector.memset(U[:], 0.0)
            Upe = psum.tile([128, 1], f32)       # PSUM: top-tree matmul col
            nc.vector.memset(Upe[:], 0.0)
        else:
            U = psum.tile([128, 64], f32)
            nc.vector.memset(U[:], 0.0)
            Upe = None

        q = state.tile([128, 64], f32, tag="q")      # O_prev
        nc.vector.memset(q[:], 0.0)
        ii = state.tile([128, 64], f32, tag="i")     # I_prev
        nc.vector.memset(ii[:], 0.0)

        lat_t = None
        for k in range(nss):
            c = k % CH
            if c == 0:
                j = k // CH
                lat_t = latp.tile([128, CH, 64], f32, tag="lat")
                src = ld_d[j * CH:(j + 1) * CH, :].rearrange(
                    "c (p f) -> p c f", p=128)
                nc.sync.dma_start(lat_t[:], src)

            # --- gather: up = sum of children outflows (prev superstep) ---
            pe_dst = Upe[0:127, 0:1] if split_u else U[0:127, 63:64]
            nc.tensor.matmul(pe_dst, amm[:, :], q[:, 63:64],
                             start=True, stop=False)
            nc.tensor.matmul(pe_dst, bmm[:, :], q[:, 0:1],
                             start=False, stop=True)
            nc.vector.tensor_add(U[:, 0:31], q[:, 1:62:2], q[:, 2:63:2])

            i_new = work.tile([128, 64], f32, tag="i_new")
            if split_u:
                nc.vector.tensor_add(i_new[:, 0:63], lat_t[:, c, 0:63],
                                     U[:, 0:63])
                nc.vector.tensor_add(i_new[:, 63:64], lat_t[:, c, 63:64],
                                     Upe[:, 0:1])
            else:
                nc.vector.tensor_add(i_new[:], lat_t[:, c, :], U[:])

            # --- MC coefficients ---
            t1 = work.tile([128, 64], f32, tag="t1")
            nc.vector.tensor_add(t1[:], i_new[:], q[:])
            qr2 = work.tile([128, 64], f32, tag="qr2")
            nc.vector.tensor_scalar(qr2[:], t1[:], 2e-3, None, op0=OP.max)
            lq2 = work.tile([128, 64], f32, tag="lq2")
            nc.scalar.activation(lq2[:], qr2[:], AF.Ln)

            g = work.tile([128, 2, 64], f32, tag="g")
            nc.vector.tensor_tensor(
                g[:], lq2[:, None, :].broadcast_to([128, 2, 64]),
                acat[:].rearrange("p (c f) -> p c f", c=2), OP.mult)
            h = work.tile([128, 2, 64], f32, tag="h")
            nc.vector.tensor_tensor(
                h[:], g[:], pcat[:].rearrange("p (c f) -> p c f", c=2),
                OP.add)
            e = work.tile([128, 2, 64], f32, tag="e")
            nc.scalar.activation(e[:], h[:], AF.Exp)
            K = e[:, 0, :]
            tt = e[:, 1, :]

            xc = work.tile([128, 64], f32, tag="xc")
            nc.vector.tensor_scalar(xc[:], tt, 0.5, None, op0=OP.min)
            y = work.tile([128, 64], f32, tag="y")
            nc.vector.scalar_tensor_tensor(y[:], K, 2.0, xc[:],
                                           op0=OP.mult, op1=OP.mult)
            den = work.tile([128, 64], f32, tag="den")
            nc.vector.scalar_tensor_tensor(den[:], y[:], dtf, K,
                                           op0=OP.add, op1=OP.add)
            n1 = work.tile([128, 64], f32, tag="n1")
            nc.vector.scalar_tensor_tensor(n1[:], y[:], dtf, K,
                                           op0=OP.add, op1=OP.subtract)
            rden = work.tile([128, 64], f32, tag="rden")
            if dve_recip:
                scr = work.tile([128, 64], f32, tag="scr")
                nc.vector.reciprocal_approx_accurate(rden[:], den[:], scr[:])
            else:
                lnden = work.tile([128, 64], f32, tag="lnden")
                nc.scalar.activation(lnden[:], den[:], AF.Ln)
                nc.scalar.activation(rden[:], lnden[:], AF.Exp, scale=-1.0)

            # --- state blend ---
            d_eng = nc.gpsimd if gps_d else nc.vector
            d1 = work.tile([128, 64], f32, tag="d1")
            d_eng.tensor_sub(d1[:], i_new[:], ii[:])
            d2 = work.tile([128, 64], f32, tag="d2")
            d_eng.tensor_sub(d2[:], ii[:], q[:])
            u1 = work.tile([128, 64], f32, tag="u1")
            nc.vector.tensor_mul(u1[:], n1[:], d1[:])
            u2 = work.tile([128, 64], f32, tag="u2")
            nc.vector.scalar_tensor_tensor(u2[:], d2[:], 2.0 * dtf, u1[:],
                                           op0=OP.mult, op1=OP.add)
            v1 = work.tile([128, 64], f32, tag="v1")
            nc.vector.tensor_mul(v1[:], u2[:], rden[:])
            v2 = work.tile([128, 64], f32, tag="v2")
            nc.vector.tensor_add(v2[:], v1[:], q[:])
            o_new = state.tile([128, 64], f32, tag="q")
            nc.vector.tensor_scalar(o_new[:], v2[:], 0.0, None, op0=OP.max)

            # outlet series (root lives at (0, 63))
            nc.scalar.activation(outbuf[0:1, k:k + 1], o_new[0:1, 63:64],
                                 AF.Copy)

            q = o_new
            ii = i_new

        nc.sync.dma_start(out_d[:], outbuf[0:1, 12:nss])

    nc.compile()
    return nc


def kernel(**inputs):
    from concourse.bass_utils import run_bass_kernel_spmd

    LD, ACAT, PCAT, Amm, Bmm = _host_precompute(inputs)
    dtf = float(inputs["dt"])
    nc = _build_bass(NSS, dtf)
    in_map = {"ld": LD, "acat": ACAT, "pcat": PCAT, "amm": Amm, "bmm": Bmm}
    res = run_bass_kernel_spmd(nc, [in_map], core_ids=[0])
    out = res.results[0]["out"].reshape(-1)
    return out.astype(np.float32)


if __name__ == "__main__":
    data = np.load("/root/problem/inputs_cache.npz")
    inputs = {k: data[k] for k in data.files}
    out = kernel(**inputs)
    exp = np.load("/root/problem/expected.npy")
    err = np.abs(out - exp) / (np.abs(exp) + 1e-6)
    print("kernel[:4]", out[:4], "expected[:4]", exp[:4])
    print("max rel err", err.max())
